# revision 82
# baseline (speedup 1.0000x reference)
"""Multi-head self-attention (B=8, N=1024, C=768, H=12) on 8 Trainium2 cores.

Strategy: data parallel — one batch element per NeuronCore, no collectives.

Per-core program (x_b is [N, C], shipped pre-transposed as xT [C, N], all
matmul operands in bf16, PSUM accumulation in fp32):
  1. qkT  [o, n] = wqk[:, o].T @ xT            o in [0, 1536)   (q and k, transposed)
       q rows evicted with  *SCALE and +SCALE*bq  (k bias cancels in softmax)
  2. v    [n, o] = xT[:, n].T @ wv             (natural layout), evicted into
       vext [n, 12*65] with a ones column appended per head
  3. per head h:  S.T[m, n] = kT_h.T @ qT_h    (K=64 matmul)
       E = exp(S.T)  (ACT, no max subtraction: logits ~ N(0,1))
       U [65, n] = vext_h.T @ E  — rows 0..63 = unnormalized out.T, row 64 = Z
       1/Z via DVE reciprocal off the PSUM Z-row, gpsimd partition_broadcast,
       outcT_h [d, n] = U[0:64] * (1/Z)b  (DVE, straight from PSUM)
  4. final [n, co] = outcT[:, n].T @ wpT + pbe   (pbe = proj_b + bv @ proj_w.T)

Schedule: PE is the bottleneck (~144 us of matmul columns); everything else
is paced to keep it busy.
  - The 96 score-exp chunks (one ACT instruction each, 1038 ns) are the
    second-largest load (~100 us). They are spread via a work queue: all 16
    chunks of heads 0/1 pair with the v-projection legs in the prelude, 5
    interleave into each qk(t>=2) phase (reading the PREVIOUS iteration's
    qT/kT — qkpool is double-buffered for this), 5 into each even-AV phase
    and 6 into each odd-AV phase, matching ACT throughput to PE phase time
    everywhere so the 3-buffer score-PSUM rotation never stalls PE.
  - AV runs its two n-halves sequentially so each [65,512] PSUM accumulator
    frees (reciprocal/broadcast/multiply chain) while the other half streams.
  - Batched multi-dim DMAs (one descriptor-gen per 3 c-chunks) spread across
    the SP and ACT HWDGE queues in consumption order; dependency-free dummy
    matmuls on a zeroed tile hold the PE p-state ramp until the first
    operands land (~5 us).
  - The tail interleaves proj partials for three leg-tiles into AV(11); the
    projection uses per-leg PSUM tiles so the 3-buffer ring advances at leg
    granularity, and the last outcT chunk (c=5) is accumulated last.
"""

import os
from contextlib import ExitStack

import numpy as np

B, N, C = 8, 1024, 768
H, D = 12, 64
SCALE = D**-0.5
NCORES = 8

P = 128
CC = C // P        # 6  c-chunks
NT = N // P        # 8  n-chunks of 128
VW = H * (D + 1)   # 780: v + ones column per head

MM_MODE = os.environ.get("BASS_MM", "bf16")
PT_BUFS = int(os.environ.get("BASS_PT_BUFS", "24"))

_built = {}


def _build():
    import concourse.bass as bass  # noqa: F401
    import concourse.mybir as mybir
    import concourse.tile as tile
    from concourse import bacc

    f32 = mybir.dt.float32
    fmm = {"bf16": mybir.dt.bfloat16, "f32r": mybir.dt.float32r}[MM_MODE]
    AF = mybir.ActivationFunctionType
    ALU = mybir.AluOpType

    nc = bacc.Bacc("TRN2", target_bir_lowering=False, debug=False, num_devices=NCORES)

    xT_d = nc.dram_tensor("xT", [C, N], fmm, kind="ExternalInput").ap()
    wqk_d = nc.dram_tensor("wqk", [C, 2 * C], fmm, kind="ExternalInput").ap()
    wv_d = nc.dram_tensor("wv", [C, C], fmm, kind="ExternalInput").ap()
    wpT_d = nc.dram_tensor("wpT", [C, C], fmm, kind="ExternalInput").ap()
    bq_d = nc.dram_tensor("bq", [P, CC], f32, kind="ExternalInput").ap()
    pbe_d = nc.dram_tensor("pbe", [P, C], f32, kind="ExternalInput").ap()
    out_d = nc.dram_tensor("out", [N, C], f32, kind="ExternalOutput").ap()

    with tile.TileContext(nc) as tc, ExitStack() as ctx:
        persist = ctx.enter_context(tc.tile_pool(name="persist", bufs=1))
        qkpool = ctx.enter_context(tc.tile_pool(name="qkpool", bufs=2))
        rpool = ctx.enter_context(tc.tile_pool(name="rpool", bufs=4))
        ppool = ctx.enter_context(tc.tile_pool(name="ppool", bufs=PT_BUFS))
        wqpool = ctx.enter_context(tc.tile_pool(name="wqpool", bufs=2))
        ocpool = ctx.enter_context(tc.tile_pool(name="ocpool", bufs=1))
        ostage = ctx.enter_context(tc.tile_pool(name="ostage", bufs=10))
        ps2 = ctx.enter_context(tc.tile_pool(name="ps2", bufs=3, space="PSUM"))
        psav = ctx.enter_context(tc.tile_pool(name="psav", bufs=2, space="PSUM"))

        x_all = persist.tile([P, CC * N], fmm, name="x_all", tag="x_all")
        xv = x_all.rearrange("p (c n) -> p c n", n=N)
        vext = [
            persist.tile([P, VW], fmm, name=f"vext{i}", tag=f"vext{i}")
            for i in range(NT)
        ]
        bq_t = persist.tile([P, CC], f32, name="bq_t", tag="bq_t")
        wv_all = persist.tile([P, CC * C], fmm, name="wv_all", tag="wv_all")
        wvv = wv_all.rearrange("p (c f) -> p c f", f=C)
        wp_all = persist.tile([P, CC * C], fmm, name="wp_all", tag="wp_all")
        wpv = wp_all.rearrange("p (c f) -> p c f", f=C)
        pbe_t = persist.tile([P, C], f32, name="pbe_t", tag="pbe_t")
        outcT = [
            ocpool.tile([P, N], fmm, name=f"outcT{i}", tag=f"outcT{i}")
            for i in range(CC)
        ]

        # ---------------- startup DMAs ----------------
        # Fine-grained, spread across the SP and ACT HWDGE queues in
        # consumption order: wq0 whole (SP), x first halves (c0 leads the ACT
        # queue so it lands right after wq0), x second halves, wv. bq rides
        # SWDGE (tiny). wq(1) goes on the ACT queue tail — on SWDGE its
        # transfer would jump ahead of the x chunks.
        wq0 = wqpool.tile([P, CC * 2 * P], fmm, name="wq", tag="wq")
        wq0v = wq0.rearrange("p (c w) -> p c w", w=2 * P)
        wqsrc = wqk_d.rearrange("(c p) w -> p c w", p=P)
        xsrc = xT_d.rearrange("(c p) n -> p c n", p=P)
        wvsrc = wv_d.rearrange("(c p) f -> p c f", p=P)

        nc.sync.dma_start(wq0v[:], wqsrc[:, :, 0 : 2 * P])
        nc.gpsimd.dma_start(bq_t[:], bq_d[:])
        nc.scalar.dma_start(xv[:, 0::2, 0:512], xsrc[:, 0::2, 0:512])
        nc.sync.dma_start(xv[:, 1::2, 0:512], xsrc[:, 1::2, 0:512])
        nc.scalar.dma_start(xv[:, 0::2, 512:1024], xsrc[:, 0::2, 512:1024])
        nc.sync.dma_start(xv[:, 1::2, 512:1024], xsrc[:, 1::2, 512:1024])
        nc.scalar.dma_start(wvv[:, 0::2, :], wvsrc[:, 0::2, :])
        nc.sync.dma_start(wvv[:, 1::2, :], wvsrc[:, 1::2, :])
        wq1 = wqpool.tile([P, CC * 2 * P], fmm, name="wq", tag="wq")
        nc.scalar.dma_start(
            wq1.rearrange("p (c w) -> p c w", w=2 * P),
            wqsrc[:, :, 2 * P : 4 * P],
        )
        C_ORDER = [0, 2, 4, 1, 3, 5]  # x/wv even chunks land first

        # PE warm-up: dependency-free dummy matmuls on a zeroed tile keep the
        # tensor engine's p-state ramp running while the first DMAs land.
        warm = persist.tile([P, 512], fmm, name="warm", tag="warm")
        nc.vector.memset(warm[:], 0.0)
        wps = psav.tile([D + 1, 512], f32, name="psa", tag="psa")
        for i in range(9):
            nc.tensor.matmul(
                wps[:], warm[:, 0 : D + 1], warm[:], start=(i == 0), stop=(i == 8)
            )

        for i in range(NT):
            nc.vector.memset(
                vext[i].rearrange("p (h e) -> p h e", e=D + 1)[:, :, D : D + 1],
                1.0,
            )

        # Head FP8_HEAD's attention-weights and V go to fp8e4m3 so its AV
        # matmuls can use DoubleRow (2 m-chunks per instruction at 0.5
        # cyc/row — 4x fewer PE columns). exp(S) <= e^5.5 fits e4m3's 448
        # range; the quantization adds ~3.6%/sqrt(12) ~ 1% to the error.
        FP8_HEADS = {10: 0, 11: 1}  # head -> v8 slot
        f8 = mybir.dt.float8e4
        MMPM = mybir.MatmulPerfMode
        # dual-fp8 LdWeights caps the stationary at 64 columns per k-tile, so
        # the ones column moves to a separate DoubleRow matmul (replicated Z).
        v8_all = persist.tile([P, 2 * 4 * 2 * D], f8, name="v8", tag="v8")
        v8v = v8_all.rearrange("p (s j t e) -> p s j t e", j=4, t=2, e=D)
        ones8 = persist.tile([P, 2 * D], f8, name="ones8", tag="ones8")
        nc.vector.memset(ones8[:], 1.0)
        shift8 = persist.tile([P, 1], f32, name="shift8", tag="shift8")
        nc.vector.memset(shift8[:], -2.0)
        p8pool = ctx.enter_context(tc.tile_pool(name="p8pool", bufs=8))
        pT8 = {}  # (h, jpair) -> [P, 2N] fp8 tile

        pT = {}        # (h, mc) -> tile
        qk_queue = []  # pending score chunks: (h, mc, qT_t, kT_t)

        def pop_score(n=1):
            for _ in range(n):
                if not qk_queue:
                    return
                h, mc, qT_t, kT_t = qk_queue.pop(0)
                r0 = (h % 2) * D
                ps = ps2.tile([P, N], f32, name="ps", tag="ps")
                for nh in range(2):
                    nc.tensor.matmul(
                        ps[:, nh * 512 : (nh + 1) * 512],
                        kT_t[r0 : r0 + D, mc * P : (mc + 1) * P],
                        qT_t[r0 : r0 + D, nh * 512 : (nh + 1) * 512],
                        start=True,
                        stop=True,
                    )
                if h in FP8_HEADS:
                    if mc % 2 == 0:
                        pT8[(h, mc // 2)] = p8pool.tile(
                            [P, 2 * N], f8, name="pt8", tag="pt8"
                        )
                    # exp(S - 2): softmax is shift-invariant; keeps exp under
                    # e4m3's 448 max (head-11 logits reach 6.16)
                    nc.scalar.activation(
                        pT8[(h, mc // 2)][:, (mc % 2) * N : (mc % 2 + 1) * N],
                        ps[:],
                        AF.Exp,
                        bias=shift8[:],
                    )
                else:
                    pt = ppool.tile([P, N], fmm, name="pt", tag="pt")
                    nc.scalar.activation(pt[:], ps[:], AF.Exp)
                    pT[(h, mc)] = pt

        def load_wq(t):
            wq = wqpool.tile([P, CC * 2 * P], fmm, name="wq", tag="wq")
            nc.gpsimd.dma_start(
                wq.rearrange("p (c w) -> p c w", w=2 * P),
                wqsrc[:, :, t * 2 * P : (t + 1) * 2 * P],
            )
            return wq.rearrange("p (c w) -> p c w", w=2 * P)

        def emit_qk(t, wq=None, fills=(0, 0, 0, 0)):
            """qkv projection for heads 2t, 2t+1. nh-outer so each n-half is
            evicted as soon as both psum tiles have it. fills = score chunks
            to interleave after each (nh, wofs) matmul group."""
            if wq is None:
                wq = load_wq(t)
            qT_t = qkpool.tile([P, N], fmm, name="qT_t", tag="qT_t")
            kT_t = qkpool.tile([P, N], fmm, name="kT_t", tag="kT_t")
            pss = [ps2.tile([P, N], f32, name="ps", tag="ps") for _ in range(2)]
            g = 0
            for nh in range(2):
                s = slice(nh * 512, (nh + 1) * 512)
                for wofs in range(2):
                    for i, c in enumerate(C_ORDER):
                        nc.tensor.matmul(
                            pss[wofs][:, s],
                            wq[:, c, wofs * P : (wofs + 1) * P],
                            xv[:, c, s],
                            start=(i == 0),
                            stop=(i == CC - 1),
                        )
                    pop_score(fills[g])
                    g += 1
                nc.vector.tensor_scalar(
                    out=qT_t[:, s],
                    in0=pss[0][:, s],
                    scalar1=SCALE,
                    scalar2=bq_t[:, t : t + 1],
                    op0=ALU.mult,
                    op1=ALU.add,
                )
                nc.vector.tensor_copy(kT_t[:, s], pss[1][:, s])
            # queue this head-pair's score work (consumed over the next phases)
            for h in (2 * t, 2 * t + 1):
                for mc in range(NT):
                    qk_queue.append((h, mc, qT_t, kT_t))
            return qT_t, kT_t

        def emit_AV(h, n_fill, fillers=None, fine_norm=False, zt_ext=None, zrow=0):
            """Attention @ V for head h, nh-halves sequential so each PSUM
            accumulator frees (normalize chain) while the other streams.
            n_fill score chunks (or explicit filler thunks) interleave.
            fine_norm splits nh0's normalize into column chunks so the first
            proj c5 legs (which only need the first columns) unblock early."""
            ti, r0 = h // 2, (h % 2) * D
            psa = [
                psav.tile([D + 1, 512], f32, name="psa", tag="psa") for _ in range(2)
            ]
            zt = None
            if h in FP8_HEADS:
                # Z for both nh halves via ones-stationary DoubleRow matmuls
                # (dual-fp8 LdWeights needs 64 weight columns; Z lands
                # replicated across 64 partitions — no broadcast needed).
                # zt_ext shares one tile between AV(10) and AV(11) (different
                # partition rows), freeing a ring slot for proj fillers.
                zt = zt_ext if zt_ext is not None else ps2.tile(
                    [P, N], f32, name="ps", tag="ps"
                )
            nf = 0
            steps = NT // 2 if h in FP8_HEADS else NT
            for nh in range(2):
                for mc in range(steps):
                    if h in FP8_HEADS:
                        rhs8 = pT8[(h, mc)].rearrange("p (t n) -> p t n", n=N)[
                            :, :, nh * 512 : (nh + 1) * 512
                        ]
                        nc.tensor.matmul(
                            psa[nh][0:D, :],
                            v8v[:, FP8_HEADS[h], mc],
                            rhs8,
                            start=(mc == 0),
                            stop=(mc == steps - 1),
                            perf_mode=MMPM.DoubleRow,
                        )
                        nc.tensor.matmul(
                            zt[0:D, nh * 512 : (nh + 1) * 512],
                            ones8.rearrange("p (t m) -> p t m", m=D),
                            rhs8,
                            start=(mc == 0),
                            stop=(mc == steps - 1),
                            perf_mode=MMPM.DoubleRow,
                        )
                    else:
                        nc.tensor.matmul(
                            psa[nh][:],
                            vext[mc][:, h * (D + 1) : (h + 1) * (D + 1)],
                            pT[(h, mc)][:, nh * 512 : (nh + 1) * 512],
                            start=(mc == 0),
                            stop=(mc == steps - 1),
                        )
                    want = (nf + 1) * 2 * steps <= (nh * steps + mc + 1) * n_fill
                    if want and nf < n_fill:
                        if fillers is not None:
                            fillers[nf]()
                        else:
                            pop_score(1)
                        nf += 1
                if h in FP8_HEADS:
                    # Z is already replicated across 64 partitions: reciprocal
                    # into SBUF (no broadcast), then multiply from PSUM
                    zb = rpool.tile([D, 512], f32, name="rb", tag="rb")
                    nc.vector.reciprocal(zb[:], zt[0:D, nh * 512 : (nh + 1) * 512])
                    nc.vector.tensor_tensor(
                        out=outcT[ti][r0 : r0 + D, nh * 512 : (nh + 1) * 512],
                        in0=psa[nh][0:D, :],
                        in1=zb[:],
                        op=ALU.mult,
                    )
                    continue
                # normalize: 1/Z off the PSUM Z-row into SBUF partition 0,
                # broadcast, multiply straight from PSUM. fine_norm chunks
                # the columns (same tiles) so the first proj c5 legs unblock
                # early after the very last AV.
                rc = rpool.tile([1, 512], f32, name="rc", tag="rc")
                rb = rpool.tile([D, 512], f32, name="rb", tag="rb")
                chunks = ((0, 256), (256, 512)) if (fine_norm and nh == 0) else (
                    (0, 512),
                )
                for z0, z1 in chunks:
                    nc.vector.reciprocal(rc[:, z0:z1], psa[nh][D : D + 1, z0:z1])
                    nc.gpsimd.partition_broadcast(rb[:, z0:z1], rc[:, z0:z1])
                    nc.vector.tensor_tensor(
                        out=outcT[ti][r0 : r0 + D, nh * 512 + z0 : nh * 512 + z1],
                        in0=psa[nh][0:D, z0:z1],
                        in1=rb[:, z0:z1],
                        op=ALU.mult,
                    )
            if h not in FP8_HEADS:
                for mc in range(NT):
                    del pT[(h, mc)]

        # ---------------- prelude: qk(0), scores(0,·)+(1,0..2) ⊗ v ----------
        qT_t, kT_t = emit_qk(0, wq=wq0v)
        for it in range(NT):
            # [pop, legA, pop, legB]: constant ring slots — score tiles
            # recycle against score ACTs (2.08us < 2.77us iteration), v tiles
            # against their own fast evictions
            pop_score(1)
            ps_v = ps2.tile([P, C], f32, name="ps", tag="ps")
            for i, c in enumerate(C_ORDER):
                nc.tensor.matmul(
                    ps_v[:, 0:512],
                    xv[:, c, it * P : (it + 1) * P],
                    wvv[:, c, 0:512],
                    start=(i == 0),
                    stop=(i == CC - 1),
                )
            # legA (features 0:512 = heads 0..7) evicts while legB streams
            nc.vector.tensor_copy(
                vext[it].rearrange("p (h e) -> p h e", e=D + 1)[:, 0:8, 0:D],
                ps_v[:, 0:512].rearrange("p (h d) -> p h d", d=D),
            )
            pop_score(1)
            for i, c in enumerate(C_ORDER):
                nc.tensor.matmul(
                    ps_v[:, 512:768],
                    xv[:, c, it * P : (it + 1) * P],
                    wvv[:, c, 512:768],
                    start=(i == 0),
                    stop=(i == CC - 1),
                )
            nc.vector.tensor_copy(
                vext[it].rearrange("p (h e) -> p h e", e=D + 1)[:, 8:12, 0:D],
                ps_v[:, 512:768].rearrange("p (h d) -> p h d", d=D),
            )
            # fp8 heads' V additionally lands in the fp8 pair tiles
            for h8, slot in FP8_HEADS.items():
                nc.vector.tensor_copy(
                    v8v[:, slot, it // 2, it % 2, :],
                    ps_v[:, h8 * D : (h8 + 1) * D],
                )
        # all 16 chunks of heads 0/1 consumed in the prelude.

        # proj weights + bias: fetched during steady state (DMA slack there)
        wpsrc = wpT_d.rearrange("(c p) f -> p c f", p=P)
        for c0, c1 in ((0, 3), (3, 6)):
            nc.sync.dma_start(wpv[:, c0:c1, :], wpsrc[:, c0:c1, :])
        nc.sync.dma_start(pbe_t[:], pbe_d[:])

        # ---------------- steady state ----------------
        # Iteration order 1,2,5,3,4: the fp8-DR (short) AV(10)/AV(11)
        # phases run mid-schedule, and the full-length bf16 AV(8)/AV(9)
        # phases close the stream where they can host the last exps and the
        # deferred projection partials.
        seq = [(1, 0, 1), (2, 2, 3), (5, 4, 5), (3, 10, 11), (4, 6, 7)]
        for i, (t, ha, hb) in enumerate(seq):
            wq = wq1.rearrange("p (c w) -> p c w", w=2 * P) if i == 0 else None
            emit_qk(t, wq=wq, fills=(0, 0, 0, 0) if i == 0 else (1, 1, 1, 2))
            if ha in FP8_HEADS:
                ztail = ps2.tile([P, N], f32, name="ps", tag="ps")
                emit_AV(ha, n_fill=5, zt_ext=ztail)
                emit_AV(hb, n_fill=6, zt_ext=ztail)
            else:
                emit_AV(ha, n_fill=5)
                emit_AV(hb, n_fill=6)

        # ---------------- tail: AV(8) ⊗ (9,3..7); AV(9) ⊗ proj -------------
        emit_AV(2 * CC - 4, n_fill=5)

        # Per-leg PSUM tiles: each [128, <=512] leg completes and evicts
        # independently, so the 3-buffer ring never waits on a half-done nt.
        proj_ps = {}
        LEGS = ((0, 512), (512, 768))

        def proj_partial(nt, leg, cs, start, stop):
            o0, o1 = LEGS[leg]

            def thunk():
                if (nt, leg) not in proj_ps:
                    proj_ps[(nt, leg)] = ps2.tile(
                        [P, o1 - o0], f32, name="ps", tag="ps"
                    )
                for c in cs:
                    nc.tensor.matmul(
                        proj_ps[(nt, leg)][:],
                        outcT[c][:, nt * P : (nt + 1) * P],
                        wpv[:, c, o0:o1],
                        start=(c == cs[0]) and start,
                        stop=(c == cs[-1]) and stop,
                    )
            return thunk

        def proj_evict(nt, leg):
            o0, o1 = LEGS[leg]
            ot = ostage.tile([P, 512], f32, name="ot", tag="ot")
            nc.vector.tensor_add(
                ot[:, 0 : o1 - o0], proj_ps[(nt, leg)][:], pbe_t[:, o0:o1]
            )
            eng = nc.scalar if nt % 2 == 0 else nc.sync
            eng.dma_start(out_d[nt * P : (nt + 1) * P, o0:o1], ot[:, 0 : o1 - o0])

        # Defer exactly 3 leg-tiles' c0..4 partials into AV(11) (ring is 3
        # buffers — a 4th deferred tile would gate allocations behind the
        # deferred c5 finishes). All three are leg A of nt 0..2: their c5
        # needs only AV(11) nh0's normalize, which lands mid-phase.
        # heads 8/9 are now the last chunk: defer c=4 (not c=5)
        OTH = [0, 1, 2, 3, 5]
        emit_AV(
            2 * CC - 3,
            n_fill=6,
            fillers=[
                proj_partial(nt, leg, cs, cs[0] == 0, False)
                for nt, leg in ((0, 0), (0, 1), (1, 0))
                for cs in (OTH[:3], OTH[3:])
            ],
        )

        cs_all = OTH + [4]
        for nt, leg in ((0, 0), (0, 1), (1, 0)):
            proj_partial(nt, leg, [4], False, True)()
            proj_evict(nt, leg)
        proj_partial(1, 1, cs_all, True, True)()
        proj_evict(1, 1)
        for nt in range(2, NT):
            for leg in (0, 1):
                proj_partial(nt, leg, cs_all, True, True)()
                proj_evict(nt, leg)

    nc.compile()
    return nc


def kernel(x, qkv_w, qkv_b, proj_w, proj_b):
    from concourse.bass_utils import run_bass_kernel_spmd

    key = (MM_MODE, PT_BUFS)
    if key not in _built:
        _built[key] = _build()
    nc = _built[key]

    x = np.asarray(x, np.float32)
    qkv_w = np.asarray(qkv_w, np.float32)
    qkv_b = np.asarray(qkv_b, np.float32)
    proj_w = np.asarray(proj_w, np.float32)
    proj_b = np.asarray(proj_b, np.float32)

    if MM_MODE == "bf16":
        import ml_dtypes

        mmdt = ml_dtypes.bfloat16
    else:
        mmdt = np.float32

    wT = np.ascontiguousarray(qkv_w.T)  # [C, 3C]
    # per-t interleave: block t = [q cols t*128:(t+1)*128 | k cols same range]
    wqk = np.concatenate(
        [
            np.concatenate(
                (wT[:, t * P : (t + 1) * P], wT[:, C + t * P : C + (t + 1) * P]),
                axis=1,
            )
            for t in range(CC)
        ],
        axis=1,
    )
    wqk = np.ascontiguousarray(wqk).astype(mmdt)
    wv = np.ascontiguousarray(wT[:, 2 * C :]).astype(mmdt)
    wpT = np.ascontiguousarray(proj_w.T).astype(mmdt)
    bq = np.ascontiguousarray((SCALE * qkv_b[:C]).reshape(CC, P).T)
    pbe = proj_b + qkv_b[2 * C :] @ proj_w.T
    pbe_b = np.ascontiguousarray(np.broadcast_to(pbe, (P, C)))

    in_maps = [
        {
            "xT": np.ascontiguousarray(x[b].T).astype(mmdt),
            "wqk": wqk,
            "wv": wv,
            "wpT": wpT,
            "bq": bq,
            "pbe": pbe_b,
        }
        for b in range(B)
    ]

    trace = bool(int(os.environ.get("BASS_PROFILE", "0")))
    res = run_bass_kernel_spmd(nc, in_maps, list(range(NCORES)), trace=trace)
    return np.stack([res.results[b]["out"] for b in range(B)])


# revision 83
# speedup vs baseline: 1.0101x; 1.0101x over previous
"""Multi-head self-attention (B=8, N=1024, C=768, H=12) on 8 Trainium2 cores.

Strategy: data parallel — one batch element per NeuronCore, no collectives.

Per-core program (x_b is [N, C], shipped pre-transposed as xT [C, N], all
matmul operands in bf16, PSUM accumulation in fp32):
  1. qkT  [o, n] = wqk[:, o].T @ xT            o in [0, 1536)   (q and k, transposed)
       q rows evicted with  *SCALE and +SCALE*bq  (k bias cancels in softmax)
  2. v    [n, o] = xT[:, n].T @ wv             (natural layout), evicted into
       vext [n, 12*65] with a ones column appended per head
  3. per head h:  S.T[m, n] = kT_h.T @ qT_h    (K=64 matmul)
       E = exp(S.T)  (ACT, no max subtraction: logits ~ N(0,1))
       U [65, n] = vext_h.T @ E  — rows 0..63 = unnormalized out.T, row 64 = Z
       1/Z via DVE reciprocal off the PSUM Z-row, gpsimd partition_broadcast,
       outcT_h [d, n] = U[0:64] * (1/Z)b  (DVE, straight from PSUM)
  4. final [n, co] = outcT[:, n].T @ wpT + pbe   (pbe = proj_b + bv @ proj_w.T)

Schedule: PE is the bottleneck (~144 us of matmul columns); everything else
is paced to keep it busy.
  - The 96 score-exp chunks (one ACT instruction each, 1038 ns) are the
    second-largest load (~100 us). They are spread via a work queue: all 16
    chunks of heads 0/1 pair with the v-projection legs in the prelude, 5
    interleave into each qk(t>=2) phase (reading the PREVIOUS iteration's
    qT/kT — qkpool is double-buffered for this), 5 into each even-AV phase
    and 6 into each odd-AV phase, matching ACT throughput to PE phase time
    everywhere so the 3-buffer score-PSUM rotation never stalls PE.
  - AV runs its two n-halves sequentially so each [65,512] PSUM accumulator
    frees (reciprocal/broadcast/multiply chain) while the other half streams.
  - Batched multi-dim DMAs (one descriptor-gen per 3 c-chunks) spread across
    the SP and ACT HWDGE queues in consumption order; dependency-free dummy
    matmuls on a zeroed tile hold the PE p-state ramp until the first
    operands land (~5 us).
  - The tail interleaves proj partials for three leg-tiles into AV(11); the
    projection uses per-leg PSUM tiles so the 3-buffer ring advances at leg
    granularity, and the last outcT chunk (c=5) is accumulated last.
"""

import os
from contextlib import ExitStack

import numpy as np

B, N, C = 8, 1024, 768
H, D = 12, 64
SCALE = D**-0.5
NCORES = 8

P = 128
CC = C // P        # 6  c-chunks
NT = N // P        # 8  n-chunks of 128
VW = H * (D + 1)   # 780: v + ones column per head

MM_MODE = os.environ.get("BASS_MM", "bf16")
PT_BUFS = int(os.environ.get("BASS_PT_BUFS", "24"))

_built = {}


def _build():
    import concourse.bass as bass  # noqa: F401
    import concourse.mybir as mybir
    import concourse.tile as tile
    from concourse import bacc

    f32 = mybir.dt.float32
    fmm = {"bf16": mybir.dt.bfloat16, "f32r": mybir.dt.float32r}[MM_MODE]
    AF = mybir.ActivationFunctionType
    ALU = mybir.AluOpType

    nc = bacc.Bacc("TRN2", target_bir_lowering=False, debug=False, num_devices=NCORES)

    xT_d = nc.dram_tensor("xT", [C, N], fmm, kind="ExternalInput").ap()
    wqk_d = nc.dram_tensor("wqk", [C, 2 * C], fmm, kind="ExternalInput").ap()
    wv_d = nc.dram_tensor("wv", [C, C], fmm, kind="ExternalInput").ap()
    wpT_d = nc.dram_tensor("wpT", [C, C], fmm, kind="ExternalInput").ap()
    bq_d = nc.dram_tensor("bq", [P, CC], f32, kind="ExternalInput").ap()
    pbe_d = nc.dram_tensor("pbe", [P, C], f32, kind="ExternalInput").ap()
    out_d = nc.dram_tensor("out", [N, C], f32, kind="ExternalOutput").ap()

    with tile.TileContext(nc) as tc, ExitStack() as ctx:
        persist = ctx.enter_context(tc.tile_pool(name="persist", bufs=1))
        qkpool = ctx.enter_context(tc.tile_pool(name="qkpool", bufs=2))
        rpool = ctx.enter_context(tc.tile_pool(name="rpool", bufs=4))
        ppool = ctx.enter_context(tc.tile_pool(name="ppool", bufs=PT_BUFS))
        wqpool = ctx.enter_context(tc.tile_pool(name="wqpool", bufs=2))
        ocpool = ctx.enter_context(tc.tile_pool(name="ocpool", bufs=1))
        ostage = ctx.enter_context(tc.tile_pool(name="ostage", bufs=10))
        ps2 = ctx.enter_context(tc.tile_pool(name="ps2", bufs=3, space="PSUM"))
        psav = ctx.enter_context(tc.tile_pool(name="psav", bufs=2, space="PSUM"))

        x_all = persist.tile([P, CC * N], fmm, name="x_all", tag="x_all")
        xv = x_all.rearrange("p (c n) -> p c n", n=N)
        vext = [
            persist.tile([P, VW], fmm, name=f"vext{i}", tag=f"vext{i}")
            for i in range(NT)
        ]
        bq_t = persist.tile([P, CC], f32, name="bq_t", tag="bq_t")
        wv_all = persist.tile([P, CC * C], fmm, name="wv_all", tag="wv_all")
        wvv = wv_all.rearrange("p (c f) -> p c f", f=C)
        wp_all = persist.tile([P, CC * C], fmm, name="wp_all", tag="wp_all")
        wpv = wp_all.rearrange("p (c f) -> p c f", f=C)
        pbe_t = persist.tile([P, C], f32, name="pbe_t", tag="pbe_t")
        outcT = [
            ocpool.tile([P, N], fmm, name=f"outcT{i}", tag=f"outcT{i}")
            for i in range(CC)
        ]

        # ---------------- startup DMAs ----------------
        # Fine-grained, spread across the SP and ACT HWDGE queues in
        # consumption order: wq0 whole (SP), x first halves (c0 leads the ACT
        # queue so it lands right after wq0), x second halves, wv. bq rides
        # SWDGE (tiny). wq(1) goes on the ACT queue tail — on SWDGE its
        # transfer would jump ahead of the x chunks.
        wq0 = wqpool.tile([P, CC * 2 * P], fmm, name="wq", tag="wq")
        wq0v = wq0.rearrange("p (c w) -> p c w", w=2 * P)
        wqsrc = wqk_d.rearrange("(c p) w -> p c w", p=P)
        xsrc = xT_d.rearrange("(c p) n -> p c n", p=P)
        wvsrc = wv_d.rearrange("(c p) f -> p c f", p=P)

        nc.sync.dma_start(wq0v[:], wqsrc[:, :, 0 : 2 * P])
        nc.gpsimd.dma_start(bq_t[:], bq_d[:])
        nc.scalar.dma_start(xv[:, 0::2, 0:512], xsrc[:, 0::2, 0:512])
        nc.sync.dma_start(xv[:, 1::2, 0:512], xsrc[:, 1::2, 0:512])
        nc.scalar.dma_start(xv[:, 0::2, 512:1024], xsrc[:, 0::2, 512:1024])
        nc.sync.dma_start(xv[:, 1::2, 512:1024], xsrc[:, 1::2, 512:1024])
        nc.scalar.dma_start(wvv[:, 0::2, :], wvsrc[:, 0::2, :])
        nc.sync.dma_start(wvv[:, 1::2, :], wvsrc[:, 1::2, :])
        wq1 = wqpool.tile([P, CC * 2 * P], fmm, name="wq", tag="wq")
        nc.scalar.dma_start(
            wq1.rearrange("p (c w) -> p c w", w=2 * P),
            wqsrc[:, :, 2 * P : 4 * P],
        )
        C_ORDER = [0, 2, 4, 1, 3, 5]  # x/wv even chunks land first

        # PE warm-up: dependency-free dummy matmuls on a zeroed tile keep the
        # tensor engine's p-state ramp running while the first DMAs land.
        warm = persist.tile([P, 512], fmm, name="warm", tag="warm")
        nc.vector.memset(warm[:], 0.0)
        wps = psav.tile([D + 1, 512], f32, name="psa", tag="psa")
        for i in range(9):
            nc.tensor.matmul(
                wps[:], warm[:, 0 : D + 1], warm[:], start=(i == 0), stop=(i == 8)
            )

        for i in range(NT):
            nc.vector.memset(
                vext[i].rearrange("p (h e) -> p h e", e=D + 1)[:, :, D : D + 1],
                1.0,
            )

        # Head FP8_HEAD's attention-weights and V go to fp8e4m3 so its AV
        # matmuls can use DoubleRow (2 m-chunks per instruction at 0.5
        # cyc/row — 4x fewer PE columns). exp(S) <= e^5.5 fits e4m3's 448
        # range; the quantization adds ~3.6%/sqrt(12) ~ 1% to the error.
        FP8_HEADS = {10: 0, 11: 1}  # head -> v8 slot
        f8 = mybir.dt.float8e4
        MMPM = mybir.MatmulPerfMode
        # dual-fp8 LdWeights caps the stationary at 64 columns per k-tile, so
        # the ones column moves to a separate DoubleRow matmul (replicated Z).
        v8_all = persist.tile([P, 2 * 4 * 2 * D], f8, name="v8", tag="v8")
        v8v = v8_all.rearrange("p (s j t e) -> p s j t e", j=4, t=2, e=D)
        ones8 = persist.tile([P, 2 * D], f8, name="ones8", tag="ones8")
        nc.vector.memset(ones8[:], 1.0)
        shift8 = persist.tile([P, 1], f32, name="shift8", tag="shift8")
        nc.vector.memset(shift8[:], -2.0)
        p8pool = ctx.enter_context(tc.tile_pool(name="p8pool", bufs=8))
        pT8 = {}  # (h, jpair) -> [P, 2N] fp8 tile

        pT = {}        # (h, mc) -> tile
        qk_queue = []  # pending score chunks: (h, mc, qT_t, kT_t)

        def pop_score(n=1):
            for _ in range(n):
                if not qk_queue:
                    return
                h, mc, qT_t, kT_t = qk_queue.pop(0)
                r0 = (h % 2) * D
                ps = ps2.tile([P, N], f32, name="ps", tag="ps")
                for nh in range(2):
                    nc.tensor.matmul(
                        ps[:, nh * 512 : (nh + 1) * 512],
                        kT_t[r0 : r0 + D, mc * P : (mc + 1) * P],
                        qT_t[r0 : r0 + D, nh * 512 : (nh + 1) * 512],
                        start=True,
                        stop=True,
                    )
                if h in FP8_HEADS:
                    if mc % 2 == 0:
                        pT8[(h, mc // 2)] = p8pool.tile(
                            [P, 2 * N], f8, name="pt8", tag="pt8"
                        )
                    # exp(S - 2): softmax is shift-invariant; keeps exp under
                    # e4m3's 448 max (head-11 logits reach 6.16)
                    nc.scalar.activation(
                        pT8[(h, mc // 2)][:, (mc % 2) * N : (mc % 2 + 1) * N],
                        ps[:],
                        AF.Exp,
                        bias=shift8[:],
                    )
                else:
                    pt = ppool.tile([P, N], fmm, name="pt", tag="pt")
                    nc.scalar.activation(pt[:], ps[:], AF.Exp)
                    pT[(h, mc)] = pt

        def load_wq(t):
            wq = wqpool.tile([P, CC * 2 * P], fmm, name="wq", tag="wq")
            nc.gpsimd.dma_start(
                wq.rearrange("p (c w) -> p c w", w=2 * P),
                wqsrc[:, :, t * 2 * P : (t + 1) * 2 * P],
            )
            return wq.rearrange("p (c w) -> p c w", w=2 * P)

        def emit_qk(t, wq=None, fills=(0, 0, 0, 0)):
            """qkv projection for heads 2t, 2t+1. nh-outer so each n-half is
            evicted as soon as both psum tiles have it. fills = score chunks
            to interleave after each (nh, wofs) matmul group."""
            if wq is None:
                wq = load_wq(t)
            qT_t = qkpool.tile([P, N], fmm, name="qT_t", tag="qT_t")
            kT_t = qkpool.tile([P, N], fmm, name="kT_t", tag="kT_t")
            pss = [ps2.tile([P, N], f32, name="ps", tag="ps") for _ in range(2)]
            g = 0
            for nh in range(2):
                s = slice(nh * 512, (nh + 1) * 512)
                for wofs in range(2):
                    for i, c in enumerate(C_ORDER):
                        nc.tensor.matmul(
                            pss[wofs][:, s],
                            wq[:, c, wofs * P : (wofs + 1) * P],
                            xv[:, c, s],
                            start=(i == 0),
                            stop=(i == CC - 1),
                        )
                    pop_score(fills[g])
                    g += 1
                nc.vector.tensor_scalar(
                    out=qT_t[:, s],
                    in0=pss[0][:, s],
                    scalar1=SCALE,
                    scalar2=bq_t[:, t : t + 1],
                    op0=ALU.mult,
                    op1=ALU.add,
                )
                nc.vector.tensor_copy(kT_t[:, s], pss[1][:, s])
            # queue this head-pair's score work (consumed over the next phases)
            for h in (2 * t, 2 * t + 1):
                for mc in range(NT):
                    qk_queue.append((h, mc, qT_t, kT_t))
            return qT_t, kT_t

        def emit_AV(h, n_fill, fillers=None, fine_norm=False, zt_ext=None, zrow=0):
            """Attention @ V for head h, nh-halves sequential so each PSUM
            accumulator frees (normalize chain) while the other streams.
            n_fill score chunks (or explicit filler thunks) interleave.
            fine_norm splits nh0's normalize into column chunks so the first
            proj c5 legs (which only need the first columns) unblock early."""
            ti, r0 = h // 2, (h % 2) * D
            psa = [
                psav.tile([D + 1, 512], f32, name="psa", tag="psa") for _ in range(2)
            ]
            zt = None
            if h in FP8_HEADS:
                # Z for both nh halves via ones-stationary DoubleRow matmuls
                # (dual-fp8 LdWeights needs 64 weight columns; Z lands
                # replicated across 64 partitions — no broadcast needed).
                # zt_ext shares one tile between AV(10) and AV(11) (different
                # partition rows), freeing a ring slot for proj fillers.
                zt = zt_ext if zt_ext is not None else ps2.tile(
                    [P, N], f32, name="ps", tag="ps"
                )
            nf = 0
            steps = NT // 2 if h in FP8_HEADS else NT
            for nh in range(2):
                for mc in range(steps):
                    if h in FP8_HEADS:
                        rhs8 = pT8[(h, mc)].rearrange("p (t n) -> p t n", n=N)[
                            :, :, nh * 512 : (nh + 1) * 512
                        ]
                        nc.tensor.matmul(
                            psa[nh][0:D, :],
                            v8v[:, FP8_HEADS[h], mc],
                            rhs8,
                            start=(mc == 0),
                            stop=(mc == steps - 1),
                            perf_mode=MMPM.DoubleRow,
                        )
                        nc.tensor.matmul(
                            zt[0:D, nh * 512 : (nh + 1) * 512],
                            ones8.rearrange("p (t m) -> p t m", m=D),
                            rhs8,
                            start=(mc == 0),
                            stop=(mc == steps - 1),
                            perf_mode=MMPM.DoubleRow,
                        )
                    else:
                        nc.tensor.matmul(
                            psa[nh][:],
                            vext[mc][:, h * (D + 1) : (h + 1) * (D + 1)],
                            pT[(h, mc)][:, nh * 512 : (nh + 1) * 512],
                            start=(mc == 0),
                            stop=(mc == steps - 1),
                        )
                    want = (nf + 1) * 2 * steps <= (nh * steps + mc + 1) * n_fill
                    if want and nf < n_fill:
                        if fillers is not None:
                            fillers[nf]()
                        else:
                            pop_score(1)
                        nf += 1
                if h in FP8_HEADS:
                    # Z is already replicated across 64 partitions: reciprocal
                    # into SBUF (no broadcast), then multiply from PSUM
                    zb = rpool.tile([D, 512], f32, name="rb", tag="rb")
                    nc.vector.reciprocal(zb[:], zt[0:D, nh * 512 : (nh + 1) * 512])
                    nc.vector.tensor_tensor(
                        out=outcT[ti][r0 : r0 + D, nh * 512 : (nh + 1) * 512],
                        in0=psa[nh][0:D, :],
                        in1=zb[:],
                        op=ALU.mult,
                    )
                    continue
                # normalize: 1/Z off the PSUM Z-row into SBUF partition 0,
                # broadcast, multiply straight from PSUM. fine_norm chunks
                # the columns (same tiles) so the first proj c5 legs unblock
                # early after the very last AV.
                rc = rpool.tile([1, 512], f32, name="rc", tag="rc")
                rb = rpool.tile([D, 512], f32, name="rb", tag="rb")
                chunks = ((0, 256), (256, 512)) if (fine_norm and nh == 0) else (
                    (0, 512),
                )
                for z0, z1 in chunks:
                    nc.vector.reciprocal(rc[:, z0:z1], psa[nh][D : D + 1, z0:z1])
                    nc.gpsimd.partition_broadcast(rb[:, z0:z1], rc[:, z0:z1])
                    nc.vector.tensor_tensor(
                        out=outcT[ti][r0 : r0 + D, nh * 512 + z0 : nh * 512 + z1],
                        in0=psa[nh][0:D, z0:z1],
                        in1=rb[:, z0:z1],
                        op=ALU.mult,
                    )
            if h not in FP8_HEADS:
                for mc in range(NT):
                    del pT[(h, mc)]

        # ---------------- prelude: qk(0), scores(0,·)+(1,0..2) ⊗ v ----------
        qT_t, kT_t = emit_qk(0, wq=wq0v)
        for it in range(NT):
            # [pop, legA, pop, legB]: constant ring slots — score tiles
            # recycle against score ACTs (2.08us < 2.77us iteration), v tiles
            # against their own fast evictions
            pop_score(1)
            ps_v = ps2.tile([P, C], f32, name="ps", tag="ps")
            for i, c in enumerate(C_ORDER):
                nc.tensor.matmul(
                    ps_v[:, 0:512],
                    xv[:, c, it * P : (it + 1) * P],
                    wvv[:, c, 0:512],
                    start=(i == 0),
                    stop=(i == CC - 1),
                )
            # legA (features 0:512 = heads 0..7) evicts while legB streams
            nc.vector.tensor_copy(
                vext[it].rearrange("p (h e) -> p h e", e=D + 1)[:, 0:8, 0:D],
                ps_v[:, 0:512].rearrange("p (h d) -> p h d", d=D),
            )
            pop_score(1)
            for i, c in enumerate(C_ORDER):
                nc.tensor.matmul(
                    ps_v[:, 512:768],
                    xv[:, c, it * P : (it + 1) * P],
                    wvv[:, c, 512:768],
                    start=(i == 0),
                    stop=(i == CC - 1),
                )
            nc.vector.tensor_copy(
                vext[it].rearrange("p (h e) -> p h e", e=D + 1)[:, 8:12, 0:D],
                ps_v[:, 512:768].rearrange("p (h d) -> p h d", d=D),
            )
            # fp8 heads' V additionally lands in the fp8 pair tiles
            for h8, slot in FP8_HEADS.items():
                nc.vector.tensor_copy(
                    v8v[:, slot, it // 2, it % 2, :],
                    ps_v[:, h8 * D : (h8 + 1) * D],
                )
        # all 16 chunks of heads 0/1 consumed in the prelude.

        # proj weights + bias: fetched during steady state (DMA slack there)
        wpsrc = wpT_d.rearrange("(c p) f -> p c f", p=P)
        for c0, c1 in ((0, 3), (3, 6)):
            nc.sync.dma_start(wpv[:, c0:c1, :], wpsrc[:, c0:c1, :])
        nc.sync.dma_start(pbe_t[:], pbe_d[:])

        # ---------------- steady state ----------------
        for t in range(1, CC):
            wq = wq1.rearrange("p (c w) -> p c w", w=2 * P) if t == 1 else None
            fills = (0, 0, 0, 0) if t == 1 else (1, 1, 1, 2)
            emit_qk(t, wq=wq, fills=fills)
            # t=5 front-loads the last heads' exps: the fp8-DR AV(10) and
            # AV(11) phases are too short to host many score fills
            emit_AV(2 * t - 2, n_fill=7 if t == CC - 1 else 5)
            emit_AV(2 * t - 1, n_fill=6 if t == CC - 1 else 6)

        # ---------------- tail: AV(10) ⊗ (11,5..7); AV(11) ⊗ proj ----------
        ztail = ps2.tile([P, N], f32, name="ps", tag="ps")
        emit_AV(2 * CC - 2, n_fill=3, zt_ext=ztail, zrow=0)

        # Per-leg PSUM tiles: each [128, <=512] leg completes and evicts
        # independently, so the 3-buffer ring never waits on a half-done nt.
        proj_ps = {}
        LEGS = ((0, 512), (512, 768))

        def proj_partial(nt, leg, cs, start, stop):
            o0, o1 = LEGS[leg]

            def thunk():
                if (nt, leg) not in proj_ps:
                    proj_ps[(nt, leg)] = ps2.tile(
                        [P, o1 - o0], f32, name="ps", tag="ps"
                    )
                for c in cs:
                    nc.tensor.matmul(
                        proj_ps[(nt, leg)][:],
                        outcT[c][:, nt * P : (nt + 1) * P],
                        wpv[:, c, o0:o1],
                        start=(c == cs[0]) and start,
                        stop=(c == cs[-1]) and stop,
                    )
            return thunk

        def proj_evict(nt, leg):
            o0, o1 = LEGS[leg]
            ot = ostage.tile([P, 512], f32, name="ot", tag="ot")
            nc.vector.tensor_add(
                ot[:, 0 : o1 - o0], proj_ps[(nt, leg)][:], pbe_t[:, o0:o1]
            )
            eng = nc.scalar if nt % 2 == 0 else nc.sync
            eng.dma_start(out_d[nt * P : (nt + 1) * P, o0:o1], ot[:, 0 : o1 - o0])

        # Defer exactly 3 leg-tiles' c0..4 partials into AV(11) (ring is 3
        # buffers — a 4th deferred tile would gate allocations behind the
        # deferred c5 finishes). All three are leg A of nt 0..2: their c5
        # needs only AV(11) nh0's normalize, which lands mid-phase.
        # only 2 deferred leg-tiles here: the fp8 Z tile takes the third ps2
        # buffer during AV(11) (3 deferred would cycle the ring against it)
        c04 = list(range(CC - 1))
        emit_AV(
            2 * CC - 1,
            n_fill=6,
            zt_ext=ztail,
            zrow=1,
            fillers=[
                proj_partial(nt, leg, cs, cs[0] == 0, False)
                for nt, leg in ((0, 0), (0, 1), (1, 0))
                for cs in (c04[:3], c04[3:])
            ],
        )

        cs_all = list(range(CC))
        for nt, leg in ((0, 0), (0, 1), (1, 0)):
            proj_partial(nt, leg, [CC - 1], False, True)()
            proj_evict(nt, leg)
        proj_partial(1, 1, cs_all, True, True)()
        proj_evict(1, 1)
        for nt in range(2, NT):
            for leg in (0, 1):
                proj_partial(nt, leg, cs_all, True, True)()
                proj_evict(nt, leg)

    nc.compile()
    return nc


def kernel(x, qkv_w, qkv_b, proj_w, proj_b):
    from concourse.bass_utils import run_bass_kernel_spmd

    key = (MM_MODE, PT_BUFS)
    if key not in _built:
        _built[key] = _build()
    nc = _built[key]

    x = np.asarray(x, np.float32)
    qkv_w = np.asarray(qkv_w, np.float32)
    qkv_b = np.asarray(qkv_b, np.float32)
    proj_w = np.asarray(proj_w, np.float32)
    proj_b = np.asarray(proj_b, np.float32)

    if MM_MODE == "bf16":
        import ml_dtypes

        mmdt = ml_dtypes.bfloat16
    else:
        mmdt = np.float32

    wT = np.ascontiguousarray(qkv_w.T)  # [C, 3C]
    # per-t interleave: block t = [q cols t*128:(t+1)*128 | k cols same range]
    wqk = np.concatenate(
        [
            np.concatenate(
                (wT[:, t * P : (t + 1) * P], wT[:, C + t * P : C + (t + 1) * P]),
                axis=1,
            )
            for t in range(CC)
        ],
        axis=1,
    )
    wqk = np.ascontiguousarray(wqk).astype(mmdt)
    wv = np.ascontiguousarray(wT[:, 2 * C :]).astype(mmdt)
    wpT = np.ascontiguousarray(proj_w.T).astype(mmdt)
    bq = np.ascontiguousarray((SCALE * qkv_b[:C]).reshape(CC, P).T)
    pbe = proj_b + qkv_b[2 * C :] @ proj_w.T
    pbe_b = np.ascontiguousarray(np.broadcast_to(pbe, (P, C)))

    in_maps = [
        {
            "xT": np.ascontiguousarray(x[b].T).astype(mmdt),
            "wqk": wqk,
            "wv": wv,
            "wpT": wpT,
            "bq": bq,
            "pbe": pbe_b,
        }
        for b in range(B)
    ]

    trace = bool(int(os.environ.get("BASS_PROFILE", "0")))
    res = run_bass_kernel_spmd(nc, in_maps, list(range(NCORES)), trace=trace)
    return np.stack([res.results[b]["out"] for b in range(B)])
